# revision 1
# baseline (speedup 1.0000x reference)
"""Trainium2 Bass kernel for KnowledgeEmbeddings (ragged_sequence).

Contract: kernel(**inputs) takes FULL unsharded inputs (numpy), returns the
FULL [64, 320, 768] f32 output.  Internally shards batch rows over 8
NeuronCores (8 rows each), replicates embedding tables, and runs a Tile/Bass
kernel per core via run_bass_kernel_spmd.

V2: table gather accumulates into the word-emb gather via DMA CCE add
(no DVE add); LN statistics via ACT accum_out passes (Square + Copy);
per-[128,1] stat math batched per group of 4 tiles; kvalid mask folded
into rstd.
"""

import functools
import numpy as np

import concourse.bass as bass
import concourse.tile as tile
from concourse import bacc, mybir
from concourse.bass import IndirectOffsetOnAxis
from concourse.bass_utils import run_bass_kernel_spmd
from concourse.masks import make_identity

# Problem constants (hardcoded per spec nn_KnowledgeEmbeddings_80839874445880)
WORD_LEN = 256
KN_LEN = 64
VOCAB = 30522
N_ENT = 500000
HID = 768
MAX_POS = 512
N_TYPES = 2
D_ENT = 100
B = 64
SEQ = WORD_LEN + KN_LEN  # 320
EPS = 1e-12

NCORES = 8
ROWS = B // NCORES           # 8 batch rows per core
WT = ROWS * WORD_LEN // 128  # 16 word tiles per core
KT = ROWS * KN_LEN // 128    # 4 knowledge tiles per core
NIDX = 2 * WT + 2 * KT       # idx tensor columns
GRP = 4                      # tiles per stats group

f32 = mybir.dt.float32
i32 = mybir.dt.int32
AF = mybir.ActivationFunctionType
ALU = mybir.AluOpType


# ---------------------------------------------------------------- host side

def _compact(ids: np.ndarray, tts: np.ndarray):
    """Vectorized numpy mirror of reference._compact_row."""
    ids = ids.astype(np.int64)
    wmask = (ids > 0) & (ids < VOCAB)
    worder = np.argsort(~wmask, axis=1, kind="stable")[:, :WORD_LEN]
    nw = wmask.sum(1, keepdims=True)
    wvalid = np.arange(WORD_LEN)[None, :] < nw
    wid = np.where(wvalid, np.take_along_axis(ids, worder, 1), 0)
    wtt = np.where(wvalid, np.take_along_axis(tts, worder, 1), 1)
    wpos = np.where(wvalid, worder, np.arange(WORD_LEN)[None, :])

    kmask = ids >= VOCAB
    korder = np.argsort(~kmask, axis=1, kind="stable")[:, :KN_LEN]
    nk = kmask.sum(1, keepdims=True)
    kvalid = np.arange(KN_LEN)[None, :] < nk
    kid = np.where(kvalid, np.take_along_axis(ids, korder, 1) - VOCAB, 0)
    ktt = np.where(kvalid, np.take_along_axis(tts, korder, 1), 0)
    kpos = np.where(kvalid, korder, 0)
    return wid, wtt, wpos, kid, ktt, kpos, kvalid


# ------------------------------------------------------------- device side

def _gather(nc, out_ap, table_ap, idx_col, accumulate=False):
    nc.gpsimd.indirect_dma_start(
        out=out_ap, out_offset=None, in_=table_ap,
        in_offset=IndirectOffsetOnAxis(ap=idx_col, axis=0),
        compute_op=ALU.add if accumulate else ALU.bypass,
    )


def _stats(nc, pools, X, SS_col, SM_col):
    """ACT passes: SS_col <- sum(X^2), SM_col <- sum(X) (per partition)."""
    scr = pools["scr"].tile([128, HID], f32, tag="scr")
    nc.scalar.activation(scr[:], X, func=AF.Square, accum_out=SS_col)
    scr2 = pools["scr"].tile([128, HID], f32, tag="scr")
    nc.scalar.activation(scr2[:], X, func=AF.Copy, accum_out=SM_col)


def _finish_stats(nc, pools, SS, SM, n, kv=None):
    """Batched [128, n] stat math.  Returns (U, RSTD) tiles.

    U = SM/HID;  RSTD = 1/sqrt(SS/HID - U^2 + eps)  (times kv if given).
    """
    spool = pools["small"]
    U_t = spool.tile([128, GRP], f32, tag="U")
    U = U_t[:, :n]
    nc.scalar.mul(U, SM, 1.0 / HID)
    SSs_t = spool.tile([128, GRP], f32, tag="SSs")
    SSs = SSs_t[:, :n]
    nc.scalar.mul(SSs, SS, 1.0 / HID)
    USQ_t = spool.tile([128, GRP], f32, tag="USQ")
    USQ = USQ_t[:, :n]
    nc.vector.tensor_mul(USQ, U, U)
    VAR_t = spool.tile([128, GRP], f32, tag="VAR")
    VAR = VAR_t[:, :n]
    nc.vector.tensor_tensor(out=VAR, in0=SSs, in1=USQ, op=ALU.subtract)
    RSTD_t = spool.tile([128, GRP], f32, tag="RSTD")
    RSTD = RSTD_t[:, :n]
    nc.scalar.activation(RSTD, VAR, func=AF.Sqrt, bias=pools["eps"][:])
    nc.vector.reciprocal(RSTD, RSTD)
    if kv is not None:
        nc.vector.tensor_mul(RSTD, RSTD, kv)
    return U, RSTD


def _normalize(nc, X, u_col, rstd_col, gamma_b, beta_b):
    nc.vector.tensor_scalar(
        out=X, in0=X, scalar1=u_col, scalar2=rstd_col,
        op0=ALU.subtract, op1=ALU.mult,
    )
    nc.vector.tensor_mul(X, X, gamma_b)
    nc.vector.tensor_add(X, X, beta_b)


def _device_kernel(tc, aps):
    nc = tc.nc
    we, ev, tbl, kwT, gb, idx, kvf, out = (
        aps["word_emb"], aps["entity_vec"], aps["table2"], aps["ke_wT"],
        aps["gamma_beta"], aps["idx"], aps["kvalid"], aps["out"],
    )
    import contextlib
    with contextlib.ExitStack() as ctx:
        singles = ctx.enter_context(tc.tile_pool(name="singles", bufs=1))
        xpool = ctx.enter_context(tc.tile_pool(name="x", bufs=2 * GRP + 2))
        spool = ctx.enter_context(tc.tile_pool(name="small", bufs=3))
        scrpool = ctx.enter_context(tc.tile_pool(name="scr", bufs=2))
        evpool = ctx.enter_context(tc.tile_pool(name="ev", bufs=3))
        psum = ctx.enter_context(tc.tile_pool(name="psum", bufs=2, space="PSUM"))

        eps_sb = singles.tile([128, 1], f32)
        nc.vector.memset(eps_sb[:], EPS)
        pools = {"small": spool, "scr": scrpool, "eps": eps_sb}

        # --- setup (once per core) ---
        idx_sb = singles.tile([128, NIDX], i32)
        nc.sync.dma_start(idx_sb[:], idx)
        kv_sb = singles.tile([128, KT], f32)
        nc.sync.dma_start(kv_sb[:], kvf)
        kw_sb = singles.tile([128, HID], f32)
        nc.vector.memset(kw_sb[:], 0.0)
        nc.sync.dma_start(kw_sb[:D_ENT, :], kwT)
        ident = singles.tile([128, 128], f32)
        make_identity(nc, ident[:])
        gbb = singles.tile([128, 4, HID], f32)
        gb_bcast = bass.AP(tensor=gb.tensor, offset=gb.offset,
                           ap=[[0, 128]] + list(gb.ap))
        nc.gpsimd.dma_start(out=gbb[:], in_=gb_bcast)

        # --- word tiles, in groups of GRP ---
        for g0 in range(0, WT, GRP):
            n = min(GRP, WT - g0)
            SS = spool.tile([128, GRP], f32, tag="SS")
            SM = spool.tile([128, GRP], f32, tag="SM")
            Xs = []
            for i in range(n):
                t = g0 + i
                X = xpool.tile([128, HID], f32, tag="X")
                _gather(nc, X[:], we, idx_sb[:, t:t + 1])
                _gather(nc, X[:], tbl, idx_sb[:, WT + t:WT + t + 1],
                        accumulate=True)
                _stats(nc, pools, X[:], SS[:, i:i + 1], SM[:, i:i + 1])
                Xs.append(X)
            U, RSTD = _finish_stats(nc, pools, SS[:, :n], SM[:, :n], n)
            for i in range(n):
                t = g0 + i
                X = Xs[i]
                _normalize(nc, X[:], U[:, i:i + 1], RSTD[:, i:i + 1],
                           gbb[:, 0, :], gbb[:, 1, :])
                b, h = divmod(t, 2)
                r = b * SEQ + h * 128
                nc.sync.dma_start(out[r:r + 128, :], X[:])

        # --- knowledge tiles (one group of KT) ---
        SS = spool.tile([128, GRP], f32, tag="SS")
        SM = spool.tile([128, GRP], f32, tag="SM")
        Xs = []
        for c in range(KT):
            EVt = evpool.tile([128, D_ENT], f32, tag="EV")
            _gather(nc, EVt[:], ev, idx_sb[:, 2 * WT + c:2 * WT + c + 1])
            ps_t = psum.tile([D_ENT, 128], f32, tag="pst")
            nc.tensor.transpose(out=ps_t[:], in_=EVt[:], identity=ident[:])
            EVT = evpool.tile([128, 128], f32, tag="EVT")
            nc.vector.memset(EVT[:], 0.0)
            nc.scalar.copy(EVT[:D_ENT, :], ps_t[:])

            X = xpool.tile([128, HID], f32, tag="X")
            _gather(nc, X[:], tbl,
                    idx_sb[:, 2 * WT + KT + c:2 * WT + KT + c + 1])
            for half in range(2):
                pm = psum.tile([128, 384], f32, tag="mm")
                nc.tensor.matmul(
                    out=pm[:], lhsT=EVT[:],
                    rhs=kw_sb[:, 384 * half:384 * (half + 1)],
                    start=True, stop=True,
                )
                nc.vector.tensor_add(
                    X[:, 384 * half:384 * (half + 1)],
                    X[:, 384 * half:384 * (half + 1)], pm[:],
                )
            _stats(nc, pools, X[:], SS[:, c:c + 1], SM[:, c:c + 1])
            Xs.append(X)
        # rstd *= kvalid: pad rows then normalize to 0 -> output = k_beta
        U, RSTD = _finish_stats(nc, pools, SS[:, :KT], SM[:, :KT], KT,
                                kv=kv_sb[:])
        for c in range(KT):
            X = Xs[c]
            _normalize(nc, X[:], U[:, c:c + 1], RSTD[:, c:c + 1],
                       gbb[:, 2, :], gbb[:, 3, :])
            r0 = (2 * c) * SEQ + WORD_LEN
            r1 = (2 * c + 1) * SEQ + WORD_LEN
            nc.sync.dma_start(out[r0:r0 + 64, :], X[0:64, :])
            nc.sync.dma_start(out[r1:r1 + 64, :], X[64:128, :])


@functools.lru_cache(maxsize=1)
def build_program():
    nc = bacc.Bacc("TRN2", target_bir_lowering=False, debug=False,
                   enable_asserts=False)
    aps = {
        "word_emb": nc.dram_tensor("word_emb", [VOCAB, HID], f32,
                                   kind="ExternalInput").ap(),
        "entity_vec": nc.dram_tensor("entity_vec", [N_ENT, D_ENT], f32,
                                     kind="ExternalInput").ap(),
        "table2": nc.dram_tensor("table2", [2 * N_TYPES * MAX_POS, HID], f32,
                                 kind="ExternalInput").ap(),
        "ke_wT": nc.dram_tensor("ke_wT", [D_ENT, HID], f32,
                                kind="ExternalInput").ap(),
        "gamma_beta": nc.dram_tensor("gamma_beta", [4, HID], f32,
                                     kind="ExternalInput").ap(),
        "idx": nc.dram_tensor("idx", [128, NIDX], i32,
                              kind="ExternalInput").ap(),
        "kvalid": nc.dram_tensor("kvalid", [128, KT], f32,
                                 kind="ExternalInput").ap(),
        "out": nc.dram_tensor("out", [ROWS * SEQ, HID], f32,
                              kind="ExternalOutput").ap(),
    }
    with tile.TileContext(nc) as tc:
        _device_kernel(tc, aps)
    nc.compile()
    return nc


def _prepare_in_maps(inputs):
    input_ids = np.asarray(inputs["input_ids"], dtype=np.int32)
    token_type_ids = np.asarray(inputs["token_type_ids"], dtype=np.int32)
    word_emb = np.ascontiguousarray(np.asarray(inputs["word_emb"], np.float32))
    pos_emb = np.asarray(inputs["pos_emb"], np.float32)
    tt_emb = np.asarray(inputs["tt_emb"], np.float32)
    entity_vec = np.ascontiguousarray(np.asarray(inputs["entityVec"], np.float32))
    ke_w = np.asarray(inputs["ke_w"], np.float32)
    ke_b = np.asarray(inputs["ke_b"], np.float32)

    # fused side table: rows [tt*512 + pos] = pos_emb[pos] + tt_emb[tt],
    # second half additionally + ke_b (knowledge branch folds its bias in)
    base = (tt_emb[:, None, :] + pos_emb[None, :, :]).reshape(
        N_TYPES * MAX_POS, HID)
    table2 = np.ascontiguousarray(
        np.concatenate([base, base + ke_b[None, :]], axis=0))
    ke_wT = np.ascontiguousarray(ke_w.T)
    gamma_beta = np.ascontiguousarray(np.stack([
        np.asarray(inputs["w_gamma"], np.float32),
        np.asarray(inputs["w_beta"], np.float32),
        np.asarray(inputs["k_gamma"], np.float32),
        np.asarray(inputs["k_beta"], np.float32),
    ]))

    wid, wtt, wpos, kid, ktt, kpos, kvalid = _compact(input_ids, token_type_ids)
    widx = wid.astype(np.int32)
    wtidx = (wpos + MAX_POS * wtt).astype(np.int32)
    kidx = kid.astype(np.int32)
    ktidx = (N_TYPES * MAX_POS + kpos + MAX_POS * ktt).astype(np.int32)
    kvf = kvalid.astype(np.float32)

    in_maps = []
    for c in range(NCORES):
        s = slice(c * ROWS, (c + 1) * ROWS)
        idx = np.concatenate([
            widx[s].reshape(WT, 128).T,
            wtidx[s].reshape(WT, 128).T,
            kidx[s].reshape(KT, 128).T,
            ktidx[s].reshape(KT, 128).T,
        ], axis=1)
        in_maps.append({
            "word_emb": word_emb,
            "entity_vec": entity_vec,
            "table2": table2,
            "ke_wT": ke_wT,
            "gamma_beta": gamma_beta,
            "idx": np.ascontiguousarray(idx),
            "kvalid": np.ascontiguousarray(kvf[s].reshape(KT, 128).T),
        })
    return in_maps


def run(inputs, trace=False):
    """Returns (full_output [64,320,768] f32, exec_time_ns or None)."""
    nc = build_program()
    in_maps = _prepare_in_maps(inputs)
    res = run_bass_kernel_spmd(nc, in_maps, list(range(NCORES)), trace=trace)
    out = np.concatenate(
        [r["out"].reshape(ROWS, SEQ, HID) for r in res.results], axis=0)
    return out, res.exec_time_ns


def kernel(**inputs) -> np.ndarray:
    out, _ = run(inputs)
    return out



# revision 7
# speedup vs baseline: 1.2219x; 1.2219x over previous
"""Trainium2 Bass kernel for KnowledgeEmbeddings (ragged_sequence).

Contract: kernel(**inputs) takes FULL unsharded inputs (numpy), returns the
FULL [64, 320, 768] f32 output.  Internally shards batch rows over 8
NeuronCores (8 rows each), replicates embedding tables, and runs a Tile/Bass
kernel per core via run_bass_kernel_spmd.

V4: bf16 tables + bf16 output (upcast on host); word/table2 gathers via
dma_gather (one SWDGE instruction per 4-tile group, int16 wrapped indices);
entityVec gather stays per-column indirect (ids exceed int16); fused
scalar_tensor_tensor ops for table-add (+sum accum) and normalize
(2 DVE ops incl. gamma/beta); gamma/beta pre-broadcast on host.
"""

import functools
import numpy as np
import ml_dtypes

import concourse.bass as bass
import concourse.tile as tile
from concourse import bacc, mybir
from concourse.bass import IndirectOffsetOnAxis
from concourse.bass_utils import run_bass_kernel_spmd
from concourse.masks import make_identity

BF16 = ml_dtypes.bfloat16

# Problem constants (hardcoded per spec nn_KnowledgeEmbeddings_80839874445880)
WORD_LEN = 256
KN_LEN = 64
VOCAB = 30522
N_ENT = 500000
HID = 768
MAX_POS = 512
N_TYPES = 2
D_ENT = 100
B = 64
SEQ = WORD_LEN + KN_LEN  # 320
EPS = 1e-12

NCORES = 8
ROWS = B // NCORES           # 8 batch rows per core
WT = ROWS * WORD_LEN // 128  # 16 word tiles per core
KT = ROWS * KN_LEN // 128    # 4 knowledge tiles per core
GRP = 4                      # tiles per stats/gather group
NG = WT // GRP               # word groups
GIDX = GRP * 128 // 16       # idx16 columns per group (32)
NI16 = (2 * NG + 1) * GIDX   # idx16 tensor columns (288)

f32 = mybir.dt.float32
bf16 = mybir.dt.bfloat16
i32 = mybir.dt.int32
i16 = mybir.dt.int16
AF = mybir.ActivationFunctionType
ALU = mybir.AluOpType


# ---------------------------------------------------------------- host side

def _compact(ids: np.ndarray, tts: np.ndarray):
    """Vectorized numpy mirror of reference._compact_row."""
    ids = ids.astype(np.int64)
    wmask = (ids > 0) & (ids < VOCAB)
    worder = np.argsort(~wmask, axis=1, kind="stable")[:, :WORD_LEN]
    nw = wmask.sum(1, keepdims=True)
    wvalid = np.arange(WORD_LEN)[None, :] < nw
    wid = np.where(wvalid, np.take_along_axis(ids, worder, 1), 0)
    wtt = np.where(wvalid, np.take_along_axis(tts, worder, 1), 1)
    wpos = np.where(wvalid, worder, np.arange(WORD_LEN)[None, :])

    kmask = ids >= VOCAB
    korder = np.argsort(~kmask, axis=1, kind="stable")[:, :KN_LEN]
    nk = kmask.sum(1, keepdims=True)
    kvalid = np.arange(KN_LEN)[None, :] < nk
    kid = np.where(kvalid, np.take_along_axis(ids, korder, 1) - VOCAB, 0)
    ktt = np.where(kvalid, np.take_along_axis(tts, korder, 1), 0)
    kpos = np.where(kvalid, korder, 0)
    return wid, wtt, wpos, kid, ktt, kpos, kvalid


def _wrap16(flat):
    """[n] flat gather indices (j = c*128+p) -> [128, n//16] wrapped int16."""
    return np.tile(flat.reshape(-1, 16).T, (8, 1)).astype(np.int16)


# ------------------------------------------------------------- device side

def _finish_stats(nc, pools, SS, SM, n, kv=None):
    """Batched [128, n] stat math.  Returns (U, RSTD) tiles.

    U = SM/HID;  RSTD = 1/sqrt(SS/HID - U^2 + eps)  (times kv if given).
    """
    spool = pools["small"]
    U_t = spool.tile([128, GRP], f32, tag="U")
    U = U_t[:, :n]
    nc.scalar.mul(U, SM, 1.0 / HID)
    SSs_t = spool.tile([128, GRP], f32, tag="SSs")
    SSs = SSs_t[:, :n]
    nc.scalar.mul(SSs, SS, 1.0 / HID)
    VAR_t = spool.tile([128, GRP], f32, tag="VAR")
    VAR = VAR_t[:, :n]
    nc.vector.scalar_tensor_tensor(
        out=VAR, in0=U, scalar=1.0, in1=U, op0=ALU.mult, op1=ALU.mult,
    )
    nc.vector.tensor_tensor(out=VAR, in0=SSs, in1=VAR, op=ALU.subtract)
    RSTD_t = spool.tile([128, GRP], f32, tag="RSTD")
    RSTD = RSTD_t[:, :n]
    nc.scalar.activation(RSTD, VAR, func=AF.Sqrt, bias=pools["eps"][:])
    nc.vector.reciprocal(RSTD, RSTD)
    if kv is not None:
        nc.vector.tensor_mul(RSTD, RSTD, kv)
    return U, RSTD


def _normalize(nc, X, scr, u_col, rstd_col, gamma_b, beta_b, out_ap):
    """out = ((X - u) * gamma) * rstd + beta   (2 fused DVE ops)."""
    nc.vector.scalar_tensor_tensor(
        out=scr, in0=X, scalar=u_col, in1=gamma_b,
        op0=ALU.subtract, op1=ALU.mult,
    )
    nc.vector.scalar_tensor_tensor(
        out=out_ap, in0=scr, scalar=rstd_col, in1=beta_b,
        op0=ALU.mult, op1=ALU.add,
    )


def _device_kernel(tc, aps):
    nc = tc.nc
    we, ev, tbl, kwT, gbb_in, idx16_in, evidx_in, kvf, out = (
        aps["word_emb"], aps["entity_vec"], aps["table2"], aps["ke_wT"],
        aps["gamma_beta"], aps["idx16"], aps["ev_idx"], aps["kvalid"],
        aps["out"],
    )
    import contextlib
    with contextlib.ExitStack() as ctx:
        singles = ctx.enter_context(tc.tile_pool(name="singles", bufs=1))
        xpool = ctx.enter_context(tc.tile_pool(name="x", bufs=2))
        tpool = ctx.enter_context(tc.tile_pool(name="t", bufs=2))
        spool = ctx.enter_context(tc.tile_pool(name="small", bufs=3))
        scrpool = ctx.enter_context(tc.tile_pool(name="scr", bufs=3))
        evpool = ctx.enter_context(tc.tile_pool(name="ev", bufs=2))
        psum = ctx.enter_context(tc.tile_pool(name="psum", bufs=2, space="PSUM"))

        eps_sb = singles.tile([128, 1], f32)
        nc.vector.memset(eps_sb[:], EPS)
        pools = {"small": spool, "eps": eps_sb}

        # --- setup (once per core) ---
        idx16_sb = singles.tile([128, NI16], i16)
        nc.sync.dma_start(idx16_sb[:], idx16_in)
        evidx_sb = singles.tile([128, KT], i32)
        nc.sync.dma_start(evidx_sb[:], evidx_in)
        kv_sb = singles.tile([128, KT], f32)
        nc.sync.dma_start(kv_sb[:], kvf)
        kw_sb = singles.tile([128, HID], bf16)
        nc.vector.memset(kw_sb[:], 0.0)
        nc.sync.dma_start(kw_sb[:D_ENT, :], kwT)
        ident = singles.tile([128, 128], bf16)
        make_identity(nc, ident[:])
        gbb = singles.tile([128, 4, HID], bf16)
        nc.sync.dma_start(gbb[:], gbb_in)
        W_GAMMA, W_BETA, K_GAMMA, K_BETA = (gbb[:, j, :] for j in range(4))

        def gather16(dst, table, col0):
            nc.gpsimd.dma_gather(
                out_ap=dst, in_ap=table,
                idxs_ap=idx16_sb[:, col0:col0 + GIDX],
                num_idxs=GRP * 128, num_idxs_reg=GRP * 128, elem_size=HID,
            )

        # --- word tiles, in groups of GRP ---
        for g in range(NG):
            SS = spool.tile([128, GRP], f32, tag="SS")
            SM = spool.tile([128, GRP], f32, tag="SM")
            Xg = xpool.tile([128, GRP, HID], bf16, tag="X")
            Tg = tpool.tile([128, GRP, HID], bf16, tag="T")
            gather16(Xg[:], we, g * GIDX)
            gather16(Tg[:], tbl, (NG + g) * GIDX)
            for i in range(GRP):
                X = Xg[:, i, :]
                # X += T, accumulating row-sum into SM
                nc.vector.scalar_tensor_tensor(
                    out=X, in0=X, scalar=0.0, in1=Tg[:, i, :],
                    op0=ALU.add, op1=ALU.add, accum_out=SM[:, i:i + 1],
                )
                scr = scrpool.tile([128, HID], bf16, tag="scr")
                nc.scalar.activation(scr[:], X, func=AF.Square,
                                     accum_out=SS[:, i:i + 1])
            U, RSTD = _finish_stats(nc, pools, SS[:], SM[:], GRP)
            for i in range(GRP):
                t = g * GRP + i
                scr2 = scrpool.tile([128, HID], bf16, tag="scr2")
                _normalize(nc, Xg[:, i, :], scr2[:],
                           U[:, i:i + 1], RSTD[:, i:i + 1],
                           W_GAMMA, W_BETA, Xg[:, i, :])
                b, h = divmod(t, 2)
                r = b * SEQ + h * 128
                nc.sync.dma_start(out[r:r + 128, :], Xg[:, i, :])

        # --- knowledge tiles (one group of KT) ---
        SS = spool.tile([128, GRP], f32, tag="SS")
        SM2 = spool.tile([128, 2 * GRP], f32, tag="SM2")
        EVg = evpool.tile([128, KT, D_ENT], bf16, tag="EVg")
        for c in range(KT):
            nc.gpsimd.indirect_dma_start(
                out=EVg[:, c, :], out_offset=None, in_=ev,
                in_offset=IndirectOffsetOnAxis(ap=evidx_sb[:, c:c + 1], axis=0),
            )
        Tg = tpool.tile([128, GRP, HID], bf16, tag="T")
        gather16(Tg[:], tbl, 2 * NG * GIDX)
        Xg = xpool.tile([128, GRP, HID], bf16, tag="X")
        for c in range(KT):
            ps_t = psum.tile([D_ENT, 128], bf16, tag="pst")
            nc.tensor.transpose(out=ps_t[:], in_=EVg[:, c, :], identity=ident[:])
            EVT = evpool.tile([128, 128], bf16, tag="EVT")
            nc.vector.memset(EVT[:], 0.0)
            nc.scalar.copy(EVT[:D_ENT, :], ps_t[:])

            X = Xg[:, c, :]
            for half in range(2):
                sl = slice(384 * half, 384 * (half + 1))
                pm = psum.tile([128, 384], f32, tag="mm")
                nc.tensor.matmul(
                    out=pm[:], lhsT=EVT[:], rhs=kw_sb[:, sl],
                    start=True, stop=True,
                )
                # X[:, sl] = T[:, sl] + pm, accumulating half row-sum
                nc.vector.scalar_tensor_tensor(
                    out=X[:, sl], in0=Tg[:, c, sl], scalar=0.0, in1=pm[:],
                    op0=ALU.add, op1=ALU.add,
                    accum_out=SM2[:, half * GRP + c:half * GRP + c + 1],
                )
            scr = scrpool.tile([128, HID], bf16, tag="scr")
            nc.scalar.activation(scr[:], X, func=AF.Square,
                                 accum_out=SS[:, c:c + 1])
        # SM = sum of halves (first halves in cols [0,KT), second in [GRP,GRP+KT))
        SM = spool.tile([128, GRP], f32, tag="SM")
        nc.vector.tensor_tensor(out=SM[:, :KT], in0=SM2[:, :KT],
                                in1=SM2[:, GRP:GRP + KT], op=ALU.add)
        # rstd *= kvalid: pad rows then normalize to 0 -> output = k_beta
        U, RSTD = _finish_stats(nc, pools, SS[:, :KT], SM[:, :KT], KT,
                                kv=kv_sb[:])
        for c in range(KT):
            scr2 = scrpool.tile([128, HID], bf16, tag="scr2")
            _normalize(nc, Xg[:, c, :], scr2[:],
                       U[:, c:c + 1], RSTD[:, c:c + 1],
                       K_GAMMA, K_BETA, Xg[:, c, :])
            r0 = (2 * c) * SEQ + WORD_LEN
            r1 = (2 * c + 1) * SEQ + WORD_LEN
            nc.sync.dma_start(out[r0:r0 + 64, :], Xg[0:64, c, :])
            nc.sync.dma_start(out[r1:r1 + 64, :], Xg[64:128, c, :])


@functools.lru_cache(maxsize=1)
def build_program():
    nc = bacc.Bacc("TRN2", target_bir_lowering=False, debug=False,
                   enable_asserts=False)
    aps = {
        "word_emb": nc.dram_tensor("word_emb", [VOCAB, HID], bf16,
                                   kind="ExternalInput").ap(),
        "entity_vec": nc.dram_tensor("entity_vec", [N_ENT, D_ENT], bf16,
                                     kind="ExternalInput").ap(),
        "table2": nc.dram_tensor("table2", [2 * N_TYPES * MAX_POS, HID], bf16,
                                 kind="ExternalInput").ap(),
        "ke_wT": nc.dram_tensor("ke_wT", [D_ENT, HID], bf16,
                                kind="ExternalInput").ap(),
        "gamma_beta": nc.dram_tensor("gamma_beta", [128, 4, HID], bf16,
                                     kind="ExternalInput").ap(),
        "idx16": nc.dram_tensor("idx16", [128, NI16], i16,
                                kind="ExternalInput").ap(),
        "ev_idx": nc.dram_tensor("ev_idx", [128, KT], i32,
                                 kind="ExternalInput").ap(),
        "kvalid": nc.dram_tensor("kvalid", [128, KT], f32,
                                 kind="ExternalInput").ap(),
        "out": nc.dram_tensor("out", [ROWS * SEQ, HID], bf16,
                              kind="ExternalOutput").ap(),
    }
    with tile.TileContext(nc) as tc:
        _device_kernel(tc, aps)
    nc.compile()
    return nc


def _prepare_in_maps(inputs):
    input_ids = np.asarray(inputs["input_ids"], dtype=np.int32)
    token_type_ids = np.asarray(inputs["token_type_ids"], dtype=np.int32)
    word_emb = np.asarray(inputs["word_emb"], np.float32)
    pos_emb = np.asarray(inputs["pos_emb"], np.float32)
    tt_emb = np.asarray(inputs["tt_emb"], np.float32)
    entity_vec = np.asarray(inputs["entityVec"], np.float32)
    ke_w = np.asarray(inputs["ke_w"], np.float32)
    ke_b = np.asarray(inputs["ke_b"], np.float32)

    word_emb_bf = np.ascontiguousarray(word_emb.astype(BF16))
    entity_vec_bf = np.ascontiguousarray(entity_vec.astype(BF16))

    # fused side table: rows [tt*512 + pos] = pos_emb[pos] + tt_emb[tt],
    # second half additionally + ke_b (knowledge branch folds its bias in)
    base = (tt_emb[:, None, :] + pos_emb[None, :, :]).reshape(
        N_TYPES * MAX_POS, HID)
    table2 = np.ascontiguousarray(
        np.concatenate([base, base + ke_b[None, :]], axis=0).astype(BF16))
    ke_wT = np.ascontiguousarray(ke_w.T.astype(BF16))
    gamma_beta = np.ascontiguousarray(np.broadcast_to(
        np.stack([
            np.asarray(inputs["w_gamma"], np.float32),
            np.asarray(inputs["w_beta"], np.float32),
            np.asarray(inputs["k_gamma"], np.float32),
            np.asarray(inputs["k_beta"], np.float32),
        ]).astype(BF16)[None], (128, 4, HID)))

    wid, wtt, wpos, kid, ktt, kpos, kvalid = _compact(input_ids, token_type_ids)
    wtidx = wpos + MAX_POS * wtt
    ktidx = N_TYPES * MAX_POS + kpos + MAX_POS * ktt
    kvf = kvalid.astype(np.float32)

    in_maps = []
    for c in range(NCORES):
        s = slice(c * ROWS, (c + 1) * ROWS)
        wflat = wid[s].reshape(-1)          # [2048], j = tile*128 + p
        tflat = wtidx[s].reshape(-1)
        ktflat = ktidx[s].reshape(-1)       # [512]
        cols = [_wrap16(wflat[g * 512:(g + 1) * 512]) for g in range(NG)]
        cols += [_wrap16(tflat[g * 512:(g + 1) * 512]) for g in range(NG)]
        cols += [_wrap16(ktflat)]
        idx16_arr = np.concatenate(cols, axis=1)
        in_maps.append({
            "word_emb": word_emb_bf,
            "entity_vec": entity_vec_bf,
            "table2": table2,
            "ke_wT": ke_wT,
            "gamma_beta": gamma_beta,
            "idx16": np.ascontiguousarray(idx16_arr),
            "ev_idx": np.ascontiguousarray(
                kid[s].reshape(KT, 128).T.astype(np.int32)),
            "kvalid": np.ascontiguousarray(kvf[s].reshape(KT, 128).T),
        })
    return in_maps


def run(inputs, trace=False):
    """Returns (full_output [64,320,768] f32, exec_time_ns or None)."""
    nc = build_program()
    in_maps = _prepare_in_maps(inputs)
    res = run_bass_kernel_spmd(nc, in_maps, list(range(NCORES)), trace=trace)
    out = np.concatenate(
        [np.asarray(r["out"], np.float32).reshape(ROWS, SEQ, HID)
         for r in res.results], axis=0)
    return out, res.exec_time_ns


def kernel(**inputs) -> np.ndarray:
    out, _ = run(inputs)
    return out


# revision 9
# speedup vs baseline: 1.2527x; 1.0252x over previous
"""Trainium2 Bass kernel for KnowledgeEmbeddings (ragged_sequence).

Contract: kernel(**inputs) takes FULL unsharded inputs (numpy), returns the
FULL [64, 320, 768] f32 output.  Internally shards batch rows over 8
NeuronCores (8 rows each), replicates embedding tables, and runs a Tile/Bass
kernel per core via run_bass_kernel_spmd.

V5: bf16 tables + bf16 output (upcast on host); word/table2 gathers via
dma_gather (int16 wrapped indices, one SWDGE instruction per 4-tile group);
entityVec gather per-column indirect (ids exceed int16).  DVE ops chosen for
fast modes: tensor_tensor (2x_1p) for adds/gamma/beta, tensor_scalar (4x_2p)
for (x-u)*rstd and the sum-accumulate pass; Square+accum on Scalar.
Knowledge tiles interleaved into the word-group loop; kn gathers issued
first so the tensor-engine chain overlaps word gathers.
"""

import functools
import numpy as np
import ml_dtypes

import concourse.bass as bass
import concourse.tile as tile
from concourse import bacc, mybir
from concourse.bass import IndirectOffsetOnAxis
from concourse.bass_utils import run_bass_kernel_spmd
from concourse.masks import make_identity

BF16 = ml_dtypes.bfloat16

# Problem constants (hardcoded per spec nn_KnowledgeEmbeddings_80839874445880)
WORD_LEN = 256
KN_LEN = 64
VOCAB = 30522
N_ENT = 500000
HID = 768
MAX_POS = 512
N_TYPES = 2
D_ENT = 100
B = 64
SEQ = WORD_LEN + KN_LEN  # 320
EPS = 1e-12

NCORES = 8
ROWS = B // NCORES           # 8 batch rows per core
WT = ROWS * WORD_LEN // 128  # 16 word tiles per core
KT = ROWS * KN_LEN // 128    # 4 knowledge tiles per core
GRP = 4                      # tiles per stats/gather group
NG = WT // GRP               # word groups (4)
GIDX = GRP * 128 // 16       # idx16 columns per group (32)
NI16 = (2 * NG + 1) * GIDX   # idx16 tensor columns (288)

f32 = mybir.dt.float32
bf16 = mybir.dt.bfloat16
i32 = mybir.dt.int32
i16 = mybir.dt.int16
AF = mybir.ActivationFunctionType
ALU = mybir.AluOpType


# ---------------------------------------------------------------- host side

def _compact(ids: np.ndarray, tts: np.ndarray):
    """Vectorized numpy mirror of reference._compact_row."""
    ids = ids.astype(np.int64)
    wmask = (ids > 0) & (ids < VOCAB)
    worder = np.argsort(~wmask, axis=1, kind="stable")[:, :WORD_LEN]
    nw = wmask.sum(1, keepdims=True)
    wvalid = np.arange(WORD_LEN)[None, :] < nw
    wid = np.where(wvalid, np.take_along_axis(ids, worder, 1), 0)
    wtt = np.where(wvalid, np.take_along_axis(tts, worder, 1), 1)
    wpos = np.where(wvalid, worder, np.arange(WORD_LEN)[None, :])

    kmask = ids >= VOCAB
    korder = np.argsort(~kmask, axis=1, kind="stable")[:, :KN_LEN]
    nk = kmask.sum(1, keepdims=True)
    kvalid = np.arange(KN_LEN)[None, :] < nk
    kid = np.where(kvalid, np.take_along_axis(ids, korder, 1) - VOCAB, 0)
    ktt = np.where(kvalid, np.take_along_axis(tts, korder, 1), 0)
    kpos = np.where(kvalid, korder, 0)
    return wid, wtt, wpos, kid, ktt, kpos, kvalid


def _wrap16(flat):
    """[n] flat gather indices (j = c*128+p) -> [128, n//16] wrapped int16."""
    return np.tile(flat.reshape(-1, 16).T, (8, 1)).astype(np.int16)


# ------------------------------------------------------------- device side

def _finish_stats(nc, pools, SS, SM, n, kv=None):
    """Batched [128, n] stat math.  Returns (U, RSTD) tiles.

    U = SM/HID;  RSTD = 1/sqrt(SS/HID - U^2 + eps)  (times kv if given).
    """
    spool = pools["small"]
    U_t = spool.tile([128, GRP], f32, tag="U")
    U = U_t[:, :n]
    nc.scalar.mul(U, SM, 1.0 / HID)
    SSs_t = spool.tile([128, GRP], f32, tag="SSs")
    SSs = SSs_t[:, :n]
    nc.scalar.mul(SSs, SS, 1.0 / HID)
    VAR_t = spool.tile([128, GRP], f32, tag="VAR")
    VAR = VAR_t[:, :n]
    nc.vector.tensor_tensor(out=VAR, in0=U, in1=U, op=ALU.mult)
    nc.vector.tensor_tensor(out=VAR, in0=SSs, in1=VAR, op=ALU.subtract)
    RSTD_t = spool.tile([128, GRP], f32, tag="RSTD")
    RSTD = RSTD_t[:, :n]
    nc.scalar.activation(RSTD, VAR, func=AF.Sqrt, bias=pools["eps"][:])
    nc.vector.reciprocal(RSTD, RSTD)
    if kv is not None:
        nc.vector.tensor_mul(RSTD, RSTD, kv)
    return U, RSTD


def _normalize(nc, scrpool, X, u_col, rstd_col, gamma_b, beta_b, out_ap):
    """out = ((X - u) * rstd) * gamma + beta  (ts 4x + 2x TT + 2x TT)."""
    scr = scrpool.tile([128, HID], bf16, tag="nrm")
    nc.vector.tensor_scalar(out=scr[:], in0=X, scalar1=u_col,
                            scalar2=rstd_col, op0=ALU.subtract, op1=ALU.mult)
    nc.vector.tensor_tensor(out=scr[:], in0=scr[:], in1=gamma_b, op=ALU.mult)
    nc.vector.tensor_tensor(out=out_ap, in0=scr[:], in1=beta_b, op=ALU.add)


def _device_kernel(tc, aps):
    nc = tc.nc
    we, ev, tbl, kwT, gbb_in, idx16_in, evidx_in, kvf, out = (
        aps["word_emb"], aps["entity_vec"], aps["table2"], aps["ke_wT"],
        aps["gamma_beta"], aps["idx16"], aps["ev_idx"], aps["kvalid"],
        aps["out"],
    )
    import contextlib
    with contextlib.ExitStack() as ctx:
        singles = ctx.enter_context(tc.tile_pool(name="singles", bufs=1))
        xpool = ctx.enter_context(tc.tile_pool(name="x", bufs=3))
        tpool = ctx.enter_context(tc.tile_pool(name="t", bufs=3))
        spool = ctx.enter_context(tc.tile_pool(name="small", bufs=3))
        scrpool = ctx.enter_context(tc.tile_pool(name="scr", bufs=4))
        evpool = ctx.enter_context(tc.tile_pool(name="ev", bufs=2))
        psum = ctx.enter_context(tc.tile_pool(name="psum", bufs=2, space="PSUM"))

        eps_sb = singles.tile([128, 1], f32)
        nc.vector.memset(eps_sb[:], EPS)
        pools = {"small": spool, "eps": eps_sb}

        # --- setup (once per core) ---
        idx16_sb = singles.tile([128, NI16], i16)
        nc.sync.dma_start(idx16_sb[:], idx16_in)
        evidx_sb = singles.tile([128, KT], i32)
        nc.sync.dma_start(evidx_sb[:], evidx_in)
        kv_sb = singles.tile([128, KT], f32)
        nc.sync.dma_start(kv_sb[:], kvf)
        kw_sb = singles.tile([128, HID], bf16)
        nc.vector.memset(kw_sb[:], 0.0)
        nc.sync.dma_start(kw_sb[:D_ENT, :], kwT)
        ident = singles.tile([128, 128], bf16)
        make_identity(nc, ident[:])
        gbb = singles.tile([128, 4, HID], bf16)
        nc.sync.dma_start(gbb[:], gbb_in)
        W_GAMMA, W_BETA, K_GAMMA, K_BETA = (gbb[:, j, :] for j in range(4))

        def gather16(dst, table, col0):
            nc.gpsimd.dma_gather(
                out_ap=dst, in_ap=table,
                idxs_ap=idx16_sb[:, col0:col0 + GIDX],
                num_idxs=GRP * 128, num_idxs_reg=GRP * 128, elem_size=HID,
            )

        # --- knowledge gathers first: unblock the PE chain early ---
        KSS = spool.tile([128, GRP], f32, tag="KSS")
        KSM = spool.tile([128, GRP], f32, tag="KSM")
        EVg = evpool.tile([128, KT, D_ENT], bf16, tag="EVg")
        for c in range(KT):
            nc.gpsimd.indirect_dma_start(
                out=EVg[:, c, :], out_offset=None, in_=ev,
                in_offset=IndirectOffsetOnAxis(ap=evidx_sb[:, c:c + 1], axis=0),
            )
        KTg = tpool.tile([128, GRP, HID], bf16, tag="KT")
        gather16(KTg[:], tbl, 2 * NG * GIDX)
        KXg = xpool.tile([128, GRP, HID], bf16, tag="KX")

        def kn_tile(c):
            ps_t = psum.tile([D_ENT, 128], bf16, tag="pst")
            nc.tensor.transpose(out=ps_t[:], in_=EVg[:, c, :],
                                identity=ident[:])
            EVT = evpool.tile([128, 128], bf16, tag="EVT")
            nc.vector.memset(EVT[:], 0.0)
            nc.scalar.copy(EVT[:D_ENT, :], ps_t[:])

            X = KXg[:, c, :]
            for half in range(2):
                sl = slice(384 * half, 384 * (half + 1))
                pm = psum.tile([128, 384], f32, tag="mm")
                nc.tensor.matmul(
                    out=pm[:], lhsT=EVT[:], rhs=kw_sb[:, sl],
                    start=True, stop=True,
                )
                nc.vector.tensor_tensor(out=X[:, sl], in0=KTg[:, c, sl],
                                        in1=pm[:], op=ALU.add)
            # row-sum (4x ts pass) + row-sum-of-squares (Scalar)
            nc.vector.tensor_scalar(out=X, in0=X, scalar1=0.0, scalar2=0.0,
                                    op0=ALU.add, op1=ALU.add,
                                    accum_out=KSM[:, c:c + 1])
            scr = scrpool.tile([128, HID], bf16, tag="sq")
            nc.scalar.activation(scr[:], X, func=AF.Square,
                                 accum_out=KSS[:, c:c + 1])

        # --- word tiles in groups of GRP, kn tile c folded after group c ---
        for g in range(NG):
            SS = spool.tile([128, GRP], f32, tag="SS")
            SM = spool.tile([128, GRP], f32, tag="SM")
            Xg = xpool.tile([128, GRP, HID], bf16, tag="X")
            Tg = tpool.tile([128, GRP, HID], bf16, tag="T")
            gather16(Xg[:], we, g * GIDX)
            gather16(Tg[:], tbl, (NG + g) * GIDX)
            for i in range(GRP):
                X = Xg[:, i, :]
                nc.vector.tensor_tensor(out=X, in0=X, in1=Tg[:, i, :],
                                        op=ALU.add)
                nc.vector.tensor_scalar(out=X, in0=X, scalar1=0.0,
                                        scalar2=0.0, op0=ALU.add,
                                        op1=ALU.add,
                                        accum_out=SM[:, i:i + 1])
                scr = scrpool.tile([128, HID], bf16, tag="sq")
                nc.scalar.activation(scr[:], X, func=AF.Square,
                                     accum_out=SS[:, i:i + 1])
            U, RSTD = _finish_stats(nc, pools, SS[:], SM[:], GRP)
            for i in range(GRP):
                t = g * GRP + i
                _normalize(nc, scrpool, Xg[:, i, :],
                           U[:, i:i + 1], RSTD[:, i:i + 1],
                           W_GAMMA, W_BETA, Xg[:, i, :])
                b, h = divmod(t, 2)
                r = b * SEQ + h * 128
                nc.sync.dma_start(out[r:r + 128, :], Xg[:, i, :])
            kn_tile(g)

        # --- knowledge finish: rstd *= kvalid (pad rows -> output = k_beta)
        U, RSTD = _finish_stats(nc, pools, KSS[:, :KT], KSM[:, :KT], KT,
                                kv=kv_sb[:])
        for c in range(KT):
            _normalize(nc, scrpool, KXg[:, c, :],
                       U[:, c:c + 1], RSTD[:, c:c + 1],
                       K_GAMMA, K_BETA, KXg[:, c, :])
            r0 = (2 * c) * SEQ + WORD_LEN
            r1 = (2 * c + 1) * SEQ + WORD_LEN
            nc.sync.dma_start(out[r0:r0 + 64, :], KXg[0:64, c, :])
            nc.sync.dma_start(out[r1:r1 + 64, :], KXg[64:128, c, :])


@functools.lru_cache(maxsize=1)
def build_program():
    nc = bacc.Bacc("TRN2", target_bir_lowering=False, debug=False,
                   enable_asserts=False)
    aps = {
        "word_emb": nc.dram_tensor("word_emb", [VOCAB, HID], bf16,
                                   kind="ExternalInput").ap(),
        "entity_vec": nc.dram_tensor("entity_vec", [N_ENT, D_ENT], bf16,
                                     kind="ExternalInput").ap(),
        "table2": nc.dram_tensor("table2", [2 * N_TYPES * MAX_POS, HID], bf16,
                                 kind="ExternalInput").ap(),
        "ke_wT": nc.dram_tensor("ke_wT", [D_ENT, HID], bf16,
                                kind="ExternalInput").ap(),
        "gamma_beta": nc.dram_tensor("gamma_beta", [128, 4, HID], bf16,
                                     kind="ExternalInput").ap(),
        "idx16": nc.dram_tensor("idx16", [128, NI16], i16,
                                kind="ExternalInput").ap(),
        "ev_idx": nc.dram_tensor("ev_idx", [128, KT], i32,
                                 kind="ExternalInput").ap(),
        "kvalid": nc.dram_tensor("kvalid", [128, KT], f32,
                                 kind="ExternalInput").ap(),
        "out": nc.dram_tensor("out", [ROWS * SEQ, HID], bf16,
                              kind="ExternalOutput").ap(),
    }
    with tile.TileContext(nc) as tc:
        _device_kernel(tc, aps)
    nc.compile()
    return nc


def _prepare_in_maps(inputs):
    input_ids = np.asarray(inputs["input_ids"], dtype=np.int32)
    token_type_ids = np.asarray(inputs["token_type_ids"], dtype=np.int32)
    word_emb = np.asarray(inputs["word_emb"], np.float32)
    pos_emb = np.asarray(inputs["pos_emb"], np.float32)
    tt_emb = np.asarray(inputs["tt_emb"], np.float32)
    entity_vec = np.asarray(inputs["entityVec"], np.float32)
    ke_w = np.asarray(inputs["ke_w"], np.float32)
    ke_b = np.asarray(inputs["ke_b"], np.float32)

    word_emb_bf = np.ascontiguousarray(word_emb.astype(BF16))
    entity_vec_bf = np.ascontiguousarray(entity_vec.astype(BF16))

    # fused side table: rows [tt*512 + pos] = pos_emb[pos] + tt_emb[tt],
    # second half additionally + ke_b (knowledge branch folds its bias in)
    base = (tt_emb[:, None, :] + pos_emb[None, :, :]).reshape(
        N_TYPES * MAX_POS, HID)
    table2 = np.ascontiguousarray(
        np.concatenate([base, base + ke_b[None, :]], axis=0).astype(BF16))
    ke_wT = np.ascontiguousarray(ke_w.T.astype(BF16))
    gamma_beta = np.ascontiguousarray(np.broadcast_to(
        np.stack([
            np.asarray(inputs["w_gamma"], np.float32),
            np.asarray(inputs["w_beta"], np.float32),
            np.asarray(inputs["k_gamma"], np.float32),
            np.asarray(inputs["k_beta"], np.float32),
        ]).astype(BF16)[None], (128, 4, HID)))

    wid, wtt, wpos, kid, ktt, kpos, kvalid = _compact(input_ids, token_type_ids)
    wtidx = wpos + MAX_POS * wtt
    ktidx = N_TYPES * MAX_POS + kpos + MAX_POS * ktt
    kvf = kvalid.astype(np.float32)

    in_maps = []
    for c in range(NCORES):
        s = slice(c * ROWS, (c + 1) * ROWS)
        wflat = wid[s].reshape(-1)          # [2048], j = tile*128 + p
        tflat = wtidx[s].reshape(-1)
        ktflat = ktidx[s].reshape(-1)       # [512]
        cols = [_wrap16(wflat[g * 512:(g + 1) * 512]) for g in range(NG)]
        cols += [_wrap16(tflat[g * 512:(g + 1) * 512]) for g in range(NG)]
        cols += [_wrap16(ktflat)]
        idx16_arr = np.concatenate(cols, axis=1)
        in_maps.append({
            "word_emb": word_emb_bf,
            "entity_vec": entity_vec_bf,
            "table2": table2,
            "ke_wT": ke_wT,
            "gamma_beta": gamma_beta,
            "idx16": np.ascontiguousarray(idx16_arr),
            "ev_idx": np.ascontiguousarray(
                kid[s].reshape(KT, 128).T.astype(np.int32)),
            "kvalid": np.ascontiguousarray(kvf[s].reshape(KT, 128).T),
        })
    return in_maps


def run(inputs, trace=False):
    """Returns (full_output [64,320,768] f32, exec_time_ns or None)."""
    nc = build_program()
    in_maps = _prepare_in_maps(inputs)
    res = run_bass_kernel_spmd(nc, in_maps, list(range(NCORES)), trace=trace)
    out = np.concatenate(
        [np.asarray(r["out"], np.float32).reshape(ROWS, SEQ, HID)
         for r in res.results], axis=0)
    return out, res.exec_time_ns


def kernel(**inputs) -> np.ndarray:
    out, _ = run(inputs)
    return out
